# revision 44
# baseline (speedup 1.0000x reference)
"""Gromov-Wasserstein embedding loss kernel for 8x TRN2 NeuronCores.

Math (see reference):
  cos[i,j]  = (e1[i] . e2[j]) / (|e1[i]| |e2[j]| + eps)
  cost      = 1 - exp(cos - 1)
  d_w       = sum(cost * trans) = sum(trans) - sum(exp(cos-1) * trans)
  reg       = |E1^T E1 - I|_F^2 + |E2^T E2 - I|_F^2  (host: O(N d^2), tiny)
  out       = [d_w, reg]

Device work is only the O(N^2) term syt = sum(trans * exp(cos-1)).
Rows of trans split 8 ways (1024 rows/core); each core computes its
1024x8192 block as 64 tiles of [128, 1024] (4 PSUM slots deep) via three
engine paths that together balance ACT / DVE / Pool / PE / DMA:

  path C: PE injects ln(t) into PSUM (identity matmul) + fp8 DoubleRow
          cos matmul on top -> ACT exp(psum - 1) with accum_out gives
          sum_j t*exp(cos-1) per partition directly. (ACT)
  path B: same PSUM = ln(t) + cos, then DVE Schraudolph: bits =
          a*psum + b -> int16, bitcast to f16 ~ t*exp(cos-1)*2^S,
          then a 4x-mode DVE tensor_scalar copy with accum_out reduces
          it. (DVE only, no ACT)
  path A: PSUM = cos only; ACT exp -> bf16; DVE scalar_tensor_tensor
          (et * t8) with accum_out; trans tile shipped as e4m3*2^27
          (halves its DMA bytes). (ACT + DVE, cheap DMA)

Host: gather, row-normalize, k-tiled transpose, fp8 quantize of the
embedding tables; ln(trans) in bf16; grams + regularizer; final scaling
of the three partial-sum groups (incl. a numerically calibrated
Schraudolph bias correction).
"""

import sys

sys.path.insert(0, "/opt/trn_rl_repo")

import numpy as np

from concourse import bass, bacc, mybir
from concourse import tile
from concourse.bass_utils import run_bass_kernel_spmd

NCORES = 8
NUM = 8192
DIM = 256
SHARD = NUM // NCORES  # 1024 rows per core
TW = 1024  # tile width
NROW = SHARD // 128  # 8 row blocks
NCOL = NUM // TW  # 8 col blocks
NTILES = NROW * NCOL  # 64
CHUNK = 2048  # n2 table streamed in column chunks this wide

BF16 = mybir.dt.bfloat16
F16 = mybir.dt.float16
F32 = mybir.dt.float32
I16 = mybir.dt.int16
FP8 = mybir.dt.float8e4
NP_BF16 = mybir.dt.np(BF16)
NP_FP8 = mybir.dt.np(FP8)
NP_F16 = np.float16

AF = mybir.ActivationFunctionType
ALU = mybir.AluOpType

# --- path assignment per visit slot (identical on every core) ----------
# A: fp8-trans + ACT exp + DVE stt-accum
# B: lnt + PE lnt-inject + DVE Schraudolph + DVE 4x-mode accum
# C: lnt + PE lnt-inject + ACT exp+accum


def _make_path_pattern(na=8, nb=24, nc_=32):
    """Interleave so ACT-consumer tiles (P/C) and DVE-consumer tiles (B)
    alternate as evenly as possible."""
    assert na + nb + nc_ == NTILES
    act_tiles = []  # P/C sequence, P spread evenly
    err = 0
    for _ in range(na + nc_):
        err += na
        if err >= na + nc_:
            err -= na + nc_
            act_tiles.append("A")
        else:
            act_tiles.append("C")
    out = []
    erb = 0
    ai = 0
    for _ in range(NTILES):
        erb += nb
        if erb >= NTILES and len(out) < NTILES and (NTILES - len(out)) > 0:
            erb -= NTILES
            out.append("B")
        else:
            out.append(act_tiles[ai])
            ai += 1
    return out


PATH = _make_path_pattern()
N_A = PATH.count("A")
N_B = PATH.count("B")
N_AD = N_A  # tiles shipping fp8 trans
N_BC = NTILES - N_A  # tiles shipping bf16 ln(trans)

# Tile visit order: column-chunk-major so each 2 MiB/4 n2 table chunk is
# needed just before its first tile. TILE_ORDER[k] = (i, jc) with jc the
# 1024-wide column block.
TILE_ORDER = [
    (i, jg2 * 2 + h) for jg2 in range(4) for i in range(NROW) for h in range(2)
]

# --- Schraudolph constants (path B: y ~ t*e^(c-1) * 2^S) ----------------
S2_SHIFT = 40.0
T_CLAMP = 1e-11
LOG2E = 1.4426950408889634
SCH_A = 1024.0 * LOG2E
SCH_B = 1024.0 * (S2_SHIFT + 15.0) - SCH_A  # bits = SCH_A*ps + SCH_B

T8_SCALE = 2.0**27


def _schraudolph_mean_ratio():
    """Value-weighted bias of the device Schraudolph path, Monte-Carlo'd
    with t ~ U(0,1)/N^2 (known) and cos ~ N(0, 1/16) using the exact op
    semantics (bf16 lnt, f32 affine, trunc to int16, bitcast f16).
    Used to unbias path-B partial sums."""
    rng = np.random.default_rng(7)
    n = 4_000_000
    t = rng.random(n, dtype=np.float32) / np.float32(NUM * NUM)
    c = np.clip(rng.normal(0, 1 / 16.0, n), -1, 1).astype(np.float32)
    lnt = np.log(np.maximum(t, T_CLAMP)).astype(NP_BF16).astype(np.float32)
    ps = lnt + c
    bits = ((np.float32(SCH_A) * ps + np.float32(SCH_B)).astype(np.float32)).astype(
        np.int16
    )
    y = bits.view(NP_F16).astype(np.float64)
    true = t.astype(np.float64) * np.exp(c.astype(np.float64) - 1.0)
    return float(y.sum() / (2.0**S2_SHIFT) / true.sum())


_cached = {}


def build_program():
    nc = bacc.Bacc(None, target_bir_lowering=False)

    idn = nc.declare_dram_parameter("idn", [128, 128], BF16, isOutput=False)
    n1d = nc.declare_dram_parameter("n1d", [2, 128, SHARD], FP8, isOutput=False)
    n2d = nc.declare_dram_parameter("n2d", [2, 128, NUM], FP8, isOutput=False)
    t8d = nc.declare_dram_parameter("t8d", [max(N_AD, 1), 128, TW], FP8, isOutput=False)
    lnd = nc.declare_dram_parameter("lnd", [N_BC, 128, TW], BF16, isOutput=False)
    acco = nc.declare_dram_parameter("acc", [128, NTILES], F32, isOutput=True)

    with tile.TileContext(nc) as tc:
        with (
            tc.tile_pool(name="const", bufs=1) as constp,
            tc.tile_pool(name="tabs", bufs=1) as tabp,
            tc.tile_pool(name="accp", bufs=1) as accp,
            tc.tile_pool(name="lntp", bufs=10) as lntp,
            tc.tile_pool(name="t8p", bufs=5) as t8p,
            tc.tile_pool(name="etp", bufs=4) as etp,
            tc.tile_pool(name="i16p", bufs=4) as i16p,
            tc.tile_pool(name="junk", bufs=1) as junkp,
            tc.tile_pool(name="psp", bufs=4, space="PSUM") as psp,
        ):
            ident = constp.tile([128, 128], BF16)
            nc.sync.dma_start(out=ident[:], in_=idn[:, :])
            neg1 = constp.tile([128, 1], F32)
            nc.vector.memset(neg1[:], -1.0)

            n1s = tabp.tile([128, 2, SHARD], FP8)
            n2s = tabp.tile([128, 2, NUM], FP8)
            acc = accp.tile([128, NTILES], F32)

            junkb = junkp.tile([128, TW], BF16)  # ACT out, never read
            junka = junkp.tile([128, TW], BF16)  # DVE stt out, never read
            junkf = junkp.tile([128, 2 * TW], F16)  # Pool ts out, never read

            def load_n2_cols(c0, c1):
                for kt in range(2):
                    nc.sync.dma_start(
                        out=n2s[:, kt, c0:c1], in_=n2d[kt, :, c0:c1]
                    )

            ia = 0  # index into t8d
            ibc = 0  # index into lnd
            for t in range(NTILES):
                i, jc = TILE_ORDER[t]
                n0 = jc * TW
                path = PATH[t]

                # data tile DMA first (so tile 0's data leads the queue)
                if path == "A":
                    t8 = t8p.tile([128, TW], FP8, tag="t8", name=f"t8_{t}")
                    nc.sync.dma_start(out=t8[:], in_=t8d[ia, :, :])
                    ia += 1
                else:
                    lt = lntp.tile([128, TW], BF16, tag="lnt", name=f"ln{t}")
                    nc.sync.dma_start(out=lt[:], in_=lnd[ibc, :, :])
                    ibc += 1

                if t == 0:
                    for kt in range(2):
                        nc.sync.dma_start(out=n1s[:, kt, :], in_=n1d[kt, :, :])
                    load_n2_cols(0, TW)  # just the first tile's columns
                elif t == 1:
                    load_n2_cols(TW, CHUNK)  # rest of the first chunk
                if t % 16 == 5 and t // 16 < 3:
                    g = t // 16 + 1  # prefetch next column chunk
                    load_n2_cols(g * CHUNK, (g + 1) * CHUNK)

                ps = psp.tile([128, TW], F32, tag="ps", name=f"ps{t}")
                lhs = n1s[:, :, i * 128 : (i + 1) * 128]

                if path == "A":
                    for q in range(2):
                        c0 = q * 512
                        nc.tensor.matmul(
                            ps[:, c0 : c0 + 512],
                            lhsT=lhs,
                            rhs=n2s[:, :, n0 + c0 : n0 + c0 + 512],
                            perf_mode=mybir.MatmulPerfMode.DoubleRow,
                            start=True,
                            stop=True,
                            skip_group_check=True,
                        )
                    et = etp.tile([128, TW], BF16, tag="et", name=f"et{t}")
                    nc.scalar.activation(et[:], ps[:], AF.Exp, bias=neg1[:, 0:1])
                    nc.vector.scalar_tensor_tensor(
                        out=junka[:],
                        in0=et[:],
                        scalar=1.0,
                        in1=t8[:],
                        op0=ALU.mult,
                        op1=ALU.mult,
                        accum_out=acc[:, t : t + 1],
                    )
                else:
                    for q in range(2):
                        c0 = q * 512
                        nc.tensor.matmul(
                            ps[:, c0 : c0 + 512],
                            lhsT=ident[:],
                            rhs=lt[:, c0 : c0 + 512],
                            start=True,
                            stop=False,
                            skip_group_check=True,
                        )
                    for q in range(2):
                        c0 = q * 512
                        nc.tensor.matmul(
                            ps[:, c0 : c0 + 512],
                            lhsT=lhs,
                            rhs=n2s[:, :, n0 + c0 : n0 + c0 + 512],
                            perf_mode=mybir.MatmulPerfMode.DoubleRow,
                            start=False,
                            stop=True,
                            skip_group_check=True,
                        )
                    if path == "C":
                        nc.scalar.activation(
                            junkb[:],
                            ps[:],
                            AF.Exp,
                            bias=neg1[:, 0:1],
                            accum_out=acc[:, t : t + 1],
                        )
                    else:  # B: Schraudolph exp of (lnt + cos - 1) on DVE
                        i16 = i16p.tile([128, TW], I16, tag="i16", name=f"i16_{t}")
                        nc.vector.tensor_scalar(
                            out=i16[:],
                            in0=ps[:],
                            scalar1=SCH_A,
                            scalar2=SCH_B,
                            op0=ALU.mult,
                            op1=ALU.add,
                        )
                        nc.vector.tensor_scalar(
                            out=junkf[:, 0:TW],
                            in0=i16[:].bitcast(F16),
                            scalar1=1.0,
                            scalar2=0.0,
                            op0=ALU.mult,
                            op1=ALU.add,
                            accum_out=acc[:, t : t + 1],
                        )

            nc.sync.dma_start(out=acco[:, :], in_=acc[:])

    nc.finalize()
    return nc


def kernel(index1, index2, trans, emb1_w, emb2_w):
    # gather (identity for arange inputs, but stay correct in general)
    e1 = np.asarray(emb1_w, dtype=np.float32)[np.asarray(index1).astype(np.int64)]
    e2 = np.asarray(emb2_w, dtype=np.float32)[np.asarray(index2).astype(np.int64)]
    trans = np.ascontiguousarray(np.asarray(trans, dtype=np.float32))

    # ---- host: regularizer (exact) + sum(trans) ----------------------
    G1 = e1.T.astype(np.float64) @ e1.astype(np.float64)
    G2 = e2.T.astype(np.float64) @ e2.astype(np.float64)
    eye = np.eye(DIM, dtype=np.float64)
    reg = ((G1 - eye) ** 2).sum() + ((G2 - eye) ** 2).sum()
    st = float(trans.sum(dtype=np.float64))

    # ---- host: normalized, k-tiled transposed fp8 tables -------------
    n1 = e1 / np.sqrt((e1 * e1).sum(axis=1, keepdims=True))
    n2 = e2 / np.sqrt((e2 * e2).sum(axis=1, keepdims=True))
    n1T8 = np.ascontiguousarray(n1.T.reshape(2, 128, NUM).astype(NP_FP8))
    n2T8 = np.ascontiguousarray(n2.T.reshape(2, 128, NUM).astype(NP_FP8))

    # ---- host: per-tile trans encodings -------------------------------
    lnt_full = np.log(np.maximum(trans, T_CLAMP)).astype(NP_BF16)
    t8_full = (trans * np.float32(T8_SCALE)).astype(NP_FP8)

    if "nc" not in _cached:
        _cached["nc"] = build_program()
        _cached["ratio"] = _schraudolph_mean_ratio()
    nc = _cached["nc"]
    mean_r = _cached["ratio"]

    idn = np.eye(128, dtype=np.float32).astype(NP_BF16)
    in_maps = []
    for c in range(NCORES):
        r0 = c * SHARD
        t8_tiles = np.zeros((max(N_AD, 1), 128, TW), dtype=NP_FP8)
        ln_tiles = np.zeros((N_BC, 128, TW), dtype=NP_BF16)
        ia = ibc = 0
        for t in range(NTILES):
            i, jc = TILE_ORDER[t]
            rs = slice(r0 + i * 128, r0 + (i + 1) * 128)
            cs = slice(jc * TW, (jc + 1) * TW)
            if PATH[t] == "A":
                t8_tiles[ia] = t8_full[rs, cs]
                ia += 1
            else:
                ln_tiles[ibc] = lnt_full[rs, cs]
                ibc += 1
        in_maps.append(
            {
                "idn": idn,
                "n1d": np.ascontiguousarray(n1T8[:, :, r0 : r0 + SHARD]),
                "n2d": n2T8,
                "t8d": t8_tiles,
                "lnd": ln_tiles,
            }
        )

    res = run_bass_kernel_spmd(nc, in_maps, list(range(NCORES)))
    results = res.results

    syt = 0.0
    for c in range(NCORES):
        a = results[c]["acc"].astype(np.float64)  # [128, NTILES]
        for t in range(NTILES):
            s = a[:, t].sum()
            if PATH[t] == "A":
                syt += s / T8_SCALE
            elif PATH[t] == "B":
                syt += s / (2.0**S2_SHIFT) / mean_r
            else:
                syt += s

    d_w = st - syt
    return np.array([d_w, reg], dtype=np.float32)


# revision 54
# speedup vs baseline: 1.0132x; 1.0132x over previous
"""Gromov-Wasserstein embedding loss kernel for 8x TRN2 NeuronCores.

Math (see reference):
  cos[i,j]  = (e1[i] . e2[j]) / (|e1[i]| |e2[j]| + eps)
  cost      = 1 - exp(cos - 1)
  d_w       = sum(cost * trans) = sum(trans) - sum(exp(cos-1) * trans)
  reg       = |E1^T E1 - I|_F^2 + |E2^T E2 - I|_F^2  (host: O(N d^2), tiny)
  out       = [d_w, reg]

Device work is only the O(N^2) term syt = sum(trans * exp(cos-1)).
Rows of trans split 8 ways (1024 rows/core); each core computes its
1024x8192 block as 64 tiles of [128, 1024] (4 PSUM slots deep) via three
engine paths that together balance ACT / DVE / Pool / PE / DMA:

  path C: PE injects ln(t) into PSUM (identity matmul) + fp8 DoubleRow
          cos matmul on top -> ACT exp(psum - 1) with accum_out gives
          sum_j t*exp(cos-1) per partition directly. (ACT)
  path B: same PSUM = ln(t) + cos, then DVE Schraudolph: bits =
          a*psum + b -> int16, bitcast to f16 ~ t*exp(cos-1)*2^S,
          then a 4x-mode DVE tensor_scalar copy with accum_out reduces
          it. (DVE only, no ACT)
  path A: PSUM = cos only; ACT exp -> bf16; DVE scalar_tensor_tensor
          (et * t8) with accum_out; trans tile shipped as e4m3*2^27
          (halves its DMA bytes). (ACT + DVE, cheap DMA)

Host: gather, row-normalize, k-tiled transpose, fp8 quantize of the
embedding tables; ln(trans) in bf16; grams + regularizer; final scaling
of the three partial-sum groups (incl. a numerically calibrated
Schraudolph bias correction).
"""

import sys

sys.path.insert(0, "/opt/trn_rl_repo")

import numpy as np

from concourse import bass, bacc, mybir
from concourse import tile
from concourse.bass_utils import run_bass_kernel_spmd

NCORES = 8
NUM = 8192
DIM = 256
SHARD = NUM // NCORES  # 1024 rows per core
TW = 1024  # tile width
NROW = SHARD // 128  # 8 row blocks
NCOL = NUM // TW  # 8 col blocks
NTILES = NROW * NCOL  # 64
CHUNK = 2048  # n2 table streamed in column chunks this wide

BF16 = mybir.dt.bfloat16
F16 = mybir.dt.float16
F32 = mybir.dt.float32
I16 = mybir.dt.int16
FP8 = mybir.dt.float8e4
NP_BF16 = mybir.dt.np(BF16)
NP_FP8 = mybir.dt.np(FP8)
NP_F16 = np.float16

AF = mybir.ActivationFunctionType
ALU = mybir.AluOpType

# --- path assignment per visit slot (identical on every core) ----------
# A: fp8-trans + ACT exp + DVE stt-accum
# B: lnt + PE lnt-inject + DVE Schraudolph + DVE 4x-mode accum
# C: lnt + PE lnt-inject + ACT exp+accum


def _make_path_pattern(na=8, nb=24, nc_=32):
    """Interleave so ACT-consumer tiles (P/C) and DVE-consumer tiles (B)
    alternate as evenly as possible."""
    assert na + nb + nc_ == NTILES
    act_tiles = []  # P/C sequence, P spread evenly
    err = 0
    for _ in range(na + nc_):
        err += na
        if err >= na + nc_:
            err -= na + nc_
            act_tiles.append("A")
        else:
            act_tiles.append("C")
    out = []
    erb = 0
    ai = 0
    for _ in range(NTILES):
        erb += nb
        if erb >= NTILES and len(out) < NTILES and (NTILES - len(out)) > 0:
            erb -= NTILES
            out.append("B")
        else:
            out.append(act_tiles[ai])
            ai += 1
    return out


ROT = 6
PATH = _make_path_pattern()
PATH = PATH[ROT:] + PATH[:ROT]
N_A = PATH.count("A")
N_B = PATH.count("B")
N_AD = N_A  # tiles shipping fp8 trans
N_BC = NTILES - N_A  # tiles shipping bf16 ln(trans)

# Tile visit order: column-chunk-major so each 2 MiB/4 n2 table chunk is
# needed just before its first tile. TILE_ORDER[k] = (i, jc) with jc the
# 1024-wide column block.
TILE_ORDER = [
    (i, jg2 * 2 + h) for jg2 in range(4) for i in range(NROW) for h in range(2)
]

# --- Schraudolph constants (path B: y ~ t*e^(c-1) * 2^S) ----------------
S2_SHIFT = 40.0
T_CLAMP = 1e-11
LOG2E = 1.4426950408889634
SCH_A = 1024.0 * LOG2E
SCH_B = 1024.0 * (S2_SHIFT + 15.0) - SCH_A  # bits = SCH_A*ps + SCH_B

T8_SCALE = 2.0**27


def _schraudolph_mean_ratio():
    """Value-weighted bias of the device Schraudolph path, Monte-Carlo'd
    with t ~ U(0,1)/N^2 (known) and cos ~ N(0, 1/16) using the exact op
    semantics (bf16 lnt, f32 affine, trunc to int16, bitcast f16).
    Used to unbias path-B partial sums."""
    rng = np.random.default_rng(7)
    n = 4_000_000
    t = rng.random(n, dtype=np.float32) / np.float32(NUM * NUM)
    c = np.clip(rng.normal(0, 1 / 16.0, n), -1, 1).astype(np.float32)
    lnt = np.log(np.maximum(t, T_CLAMP)).astype(NP_BF16).astype(np.float32)
    ps = lnt + c
    bits = ((np.float32(SCH_A) * ps + np.float32(SCH_B)).astype(np.float32)).astype(
        np.int16
    )
    y = bits.view(NP_F16).astype(np.float64)
    true = t.astype(np.float64) * np.exp(c.astype(np.float64) - 1.0)
    return float(y.sum() / (2.0**S2_SHIFT) / true.sum())


_cached = {}


def build_program():
    nc = bacc.Bacc(None, target_bir_lowering=False)

    idn = nc.declare_dram_parameter("idn", [128, 128], BF16, isOutput=False)
    n1d = nc.declare_dram_parameter("n1d", [2, 128, SHARD], FP8, isOutput=False)
    n2d = nc.declare_dram_parameter("n2d", [2, 128, NUM], FP8, isOutput=False)
    t8d = nc.declare_dram_parameter("t8d", [max(N_AD, 1), 128, TW], FP8, isOutput=False)
    lnd = nc.declare_dram_parameter("lnd", [N_BC, 128, TW], BF16, isOutput=False)
    acco = nc.declare_dram_parameter("acc", [128, NTILES], F32, isOutput=True)

    with tile.TileContext(nc) as tc:
        with (
            tc.tile_pool(name="const", bufs=1) as constp,
            tc.tile_pool(name="tabs", bufs=1) as tabp,
            tc.tile_pool(name="accp", bufs=1) as accp,
            tc.tile_pool(name="lntp", bufs=10) as lntp,
            tc.tile_pool(name="t8p", bufs=5) as t8p,
            tc.tile_pool(name="etp", bufs=4) as etp,
            tc.tile_pool(name="i16p", bufs=4) as i16p,
            tc.tile_pool(name="junk", bufs=1) as junkp,
            tc.tile_pool(name="psp", bufs=4, space="PSUM") as psp,
        ):
            ident = constp.tile([128, 128], BF16)
            nc.sync.dma_start(out=ident[:], in_=idn[:, :])
            neg1 = constp.tile([128, 1], F32)
            nc.vector.memset(neg1[:], -1.0)

            n1s = tabp.tile([128, 2, SHARD], FP8)
            n2s = tabp.tile([128, 2, NUM], FP8)
            acc = accp.tile([128, NTILES], F32)

            junkb = junkp.tile([128, TW], BF16)  # ACT out, never read
            junka = junkp.tile([128, TW], BF16)  # DVE stt out, never read
            junkf = junkp.tile([128, 2 * TW], F16)  # Pool ts out, never read

            def load_n2_cols(c0, c1):
                for kt in range(2):
                    nc.sync.dma_start(
                        out=n2s[:, kt, c0:c1], in_=n2d[kt, :, c0:c1]
                    )

            ia = 0  # index into t8d
            ibc = 0  # index into lnd
            for t in range(NTILES):
                i, jc = TILE_ORDER[t]
                n0 = jc * TW
                path = PATH[t]

                # data tile DMA first (so tile 0's data leads the queue)
                if path == "A":
                    t8 = t8p.tile([128, TW], FP8, tag="t8", name=f"t8_{t}")
                    nc.sync.dma_start(out=t8[:], in_=t8d[ia, :, :])
                    ia += 1
                else:
                    lt = lntp.tile([128, TW], BF16, tag="lnt", name=f"ln{t}")
                    nc.sync.dma_start(out=lt[:], in_=lnd[ibc, :, :])
                    ibc += 1

                if t == 0:
                    for kt in range(2):
                        nc.sync.dma_start(out=n1s[:, kt, :], in_=n1d[kt, :, :])
                    load_n2_cols(0, TW)  # just the first tile's columns
                elif t == 1:
                    load_n2_cols(TW, CHUNK)  # rest of the first chunk
                if t % 16 == 5 and t // 16 < 3:
                    g = t // 16 + 1  # prefetch next column chunk
                    load_n2_cols(g * CHUNK, (g + 1) * CHUNK)

                ps = psp.tile([128, TW], F32, tag="ps", name=f"ps{t}")
                lhs = n1s[:, :, i * 128 : (i + 1) * 128]

                if path == "A":
                    for q in range(2):
                        c0 = q * 512
                        nc.tensor.matmul(
                            ps[:, c0 : c0 + 512],
                            lhsT=lhs,
                            rhs=n2s[:, :, n0 + c0 : n0 + c0 + 512],
                            perf_mode=mybir.MatmulPerfMode.DoubleRow,
                            start=True,
                            stop=True,
                            skip_group_check=True,
                        )
                    et = etp.tile([128, TW], BF16, tag="et", name=f"et{t}")
                    nc.scalar.activation(et[:], ps[:], AF.Exp, bias=neg1[:, 0:1])
                    nc.vector.scalar_tensor_tensor(
                        out=junka[:],
                        in0=et[:],
                        scalar=1.0,
                        in1=t8[:],
                        op0=ALU.mult,
                        op1=ALU.mult,
                        accum_out=acc[:, t : t + 1],
                    )
                else:
                    for q in range(2):
                        c0 = q * 512
                        nc.tensor.matmul(
                            ps[:, c0 : c0 + 512],
                            lhsT=ident[:],
                            rhs=lt[:, c0 : c0 + 512],
                            start=True,
                            stop=False,
                            skip_group_check=True,
                        )
                    for q in range(2):
                        c0 = q * 512
                        nc.tensor.matmul(
                            ps[:, c0 : c0 + 512],
                            lhsT=lhs,
                            rhs=n2s[:, :, n0 + c0 : n0 + c0 + 512],
                            perf_mode=mybir.MatmulPerfMode.DoubleRow,
                            start=False,
                            stop=True,
                            skip_group_check=True,
                        )
                    if path == "C":
                        nc.scalar.activation(
                            junkb[:],
                            ps[:],
                            AF.Exp,
                            bias=neg1[:, 0:1],
                            accum_out=acc[:, t : t + 1],
                        )
                    else:  # B: Schraudolph exp of (lnt + cos - 1) on DVE
                        i16 = i16p.tile([128, TW], I16, tag="i16", name=f"i16_{t}")
                        nc.vector.tensor_scalar(
                            out=i16[:],
                            in0=ps[:],
                            scalar1=SCH_A,
                            scalar2=SCH_B,
                            op0=ALU.mult,
                            op1=ALU.add,
                        )
                        nc.vector.tensor_scalar(
                            out=junkf[:, 0:TW],
                            in0=i16[:].bitcast(F16),
                            scalar1=1.0,
                            scalar2=0.0,
                            op0=ALU.mult,
                            op1=ALU.add,
                            accum_out=acc[:, t : t + 1],
                        )

            nc.sync.dma_start(out=acco[:, :], in_=acc[:])

    nc.finalize()
    return nc


def kernel(index1, index2, trans, emb1_w, emb2_w):
    # gather (identity for arange inputs, but stay correct in general)
    e1 = np.asarray(emb1_w, dtype=np.float32)[np.asarray(index1).astype(np.int64)]
    e2 = np.asarray(emb2_w, dtype=np.float32)[np.asarray(index2).astype(np.int64)]
    trans = np.ascontiguousarray(np.asarray(trans, dtype=np.float32))

    # ---- host: regularizer (exact) + sum(trans) ----------------------
    G1 = e1.T.astype(np.float64) @ e1.astype(np.float64)
    G2 = e2.T.astype(np.float64) @ e2.astype(np.float64)
    eye = np.eye(DIM, dtype=np.float64)
    reg = ((G1 - eye) ** 2).sum() + ((G2 - eye) ** 2).sum()
    st = float(trans.sum(dtype=np.float64))

    # ---- host: normalized, k-tiled transposed fp8 tables -------------
    n1 = e1 / np.sqrt((e1 * e1).sum(axis=1, keepdims=True))
    n2 = e2 / np.sqrt((e2 * e2).sum(axis=1, keepdims=True))
    n1T8 = np.ascontiguousarray(n1.T.reshape(2, 128, NUM).astype(NP_FP8))
    n2T8 = np.ascontiguousarray(n2.T.reshape(2, 128, NUM).astype(NP_FP8))

    # ---- host: per-tile trans encodings -------------------------------
    lnt_full = np.log(np.maximum(trans, T_CLAMP)).astype(NP_BF16)
    t8_full = (trans * np.float32(T8_SCALE)).astype(NP_FP8)

    if "nc" not in _cached:
        _cached["nc"] = build_program()
        _cached["ratio"] = _schraudolph_mean_ratio()
    nc = _cached["nc"]
    mean_r = _cached["ratio"]

    idn = np.eye(128, dtype=np.float32).astype(NP_BF16)
    in_maps = []
    for c in range(NCORES):
        r0 = c * SHARD
        t8_tiles = np.zeros((max(N_AD, 1), 128, TW), dtype=NP_FP8)
        ln_tiles = np.zeros((N_BC, 128, TW), dtype=NP_BF16)
        ia = ibc = 0
        for t in range(NTILES):
            i, jc = TILE_ORDER[t]
            rs = slice(r0 + i * 128, r0 + (i + 1) * 128)
            cs = slice(jc * TW, (jc + 1) * TW)
            if PATH[t] == "A":
                t8_tiles[ia] = t8_full[rs, cs]
                ia += 1
            else:
                ln_tiles[ibc] = lnt_full[rs, cs]
                ibc += 1
        in_maps.append(
            {
                "idn": idn,
                "n1d": np.ascontiguousarray(n1T8[:, :, r0 : r0 + SHARD]),
                "n2d": n2T8,
                "t8d": t8_tiles,
                "lnd": ln_tiles,
            }
        )

    res = run_bass_kernel_spmd(nc, in_maps, list(range(NCORES)))
    results = res.results

    syt = 0.0
    for c in range(NCORES):
        a = results[c]["acc"].astype(np.float64)  # [128, NTILES]
        for t in range(NTILES):
            s = a[:, t].sum()
            if PATH[t] == "A":
                syt += s / T8_SCALE
            elif PATH[t] == "B":
                syt += s / (2.0**S2_SHIFT) / mean_r
            else:
                syt += s

    d_w = st - syt
    return np.array([d_w, reg], dtype=np.float32)


# revision 62
# speedup vs baseline: 1.0153x; 1.0021x over previous
"""Gromov-Wasserstein embedding loss kernel for 8x TRN2 NeuronCores.

Math (see reference):
  cos[i,j]  = (e1[i] . e2[j]) / (|e1[i]| |e2[j]| + eps)
  cost      = 1 - exp(cos - 1)
  d_w       = sum(cost * trans) = sum(trans) - sum(exp(cos-1) * trans)
  reg       = |E1^T E1 - I|_F^2 + |E2^T E2 - I|_F^2  (host: O(N d^2), tiny)
  out       = [d_w, reg]

Device work is only the O(N^2) term syt = sum(trans * exp(cos-1)).
Rows of trans split 8 ways (1024 rows/core); each core computes its
1024x8192 block as 64 tiles of [128, 1024] (4 PSUM slots deep) via three
engine paths that together balance ACT / DVE / Pool / PE / DMA:

  path C: PE injects ln(t) into PSUM (identity matmul) + fp8 DoubleRow
          cos matmul on top -> ACT exp(psum - 1) with accum_out gives
          sum_j t*exp(cos-1) per partition directly. (ACT)
  path B: same PSUM = ln(t) + cos, then DVE Schraudolph: bits =
          a*psum + b -> int16, bitcast to f16 ~ t*exp(cos-1)*2^S,
          then a 4x-mode DVE tensor_scalar copy with accum_out reduces
          it. (DVE only, no ACT)
  path A: PSUM = cos only; ACT exp -> bf16; DVE scalar_tensor_tensor
          (et * t8) with accum_out; trans tile shipped as e4m3*2^27
          (halves its DMA bytes). (ACT + DVE, cheap DMA)

Host: gather, row-normalize, k-tiled transpose, fp8 quantize of the
embedding tables; ln(trans) in bf16; grams + regularizer; final scaling
of the three partial-sum groups (incl. a numerically calibrated
Schraudolph bias correction).
"""

import sys

sys.path.insert(0, "/opt/trn_rl_repo")

import numpy as np

from concourse import bass, bacc, mybir
from concourse import tile
from concourse.bass_utils import run_bass_kernel_spmd

NCORES = 8
NUM = 8192
DIM = 256
SHARD = NUM // NCORES  # 1024 rows per core
TW = 1024  # tile width
NROW = SHARD // 128  # 8 row blocks
NCOL = NUM // TW  # 8 col blocks
NTILES = NROW * NCOL  # 64
CHUNK = 2048  # n2 table streamed in column chunks this wide

BF16 = mybir.dt.bfloat16
F16 = mybir.dt.float16
F32 = mybir.dt.float32
I16 = mybir.dt.int16
FP8 = mybir.dt.float8e4
NP_BF16 = mybir.dt.np(BF16)
NP_FP8 = mybir.dt.np(FP8)
NP_F16 = np.float16

AF = mybir.ActivationFunctionType
ALU = mybir.AluOpType

# --- path assignment per visit slot (identical on every core) ----------
# A: fp8-trans + ACT exp + DVE stt-accum
# B: lnt + PE lnt-inject + DVE Schraudolph + DVE 4x-mode accum
# C: lnt + PE lnt-inject + ACT exp+accum


def _make_path_pattern(na=8, nb=24, nc_=32):
    """Interleave so ACT-consumer tiles (P/C) and DVE-consumer tiles (B)
    alternate as evenly as possible."""
    assert na + nb + nc_ == NTILES
    act_tiles = []  # P/C sequence, P spread evenly
    err = 0
    for _ in range(na + nc_):
        err += na
        if err >= na + nc_:
            err -= na + nc_
            act_tiles.append("A")
        else:
            act_tiles.append("C")
    out = []
    erb = 0
    ai = 0
    for _ in range(NTILES):
        erb += nb
        if erb >= NTILES and len(out) < NTILES and (NTILES - len(out)) > 0:
            erb -= NTILES
            out.append("B")
        else:
            out.append(act_tiles[ai])
            ai += 1
    return out


ROT = 6
PATH = _make_path_pattern()
PATH = PATH[ROT:] + PATH[:ROT]
N_A = PATH.count("A")
N_B = PATH.count("B")
N_AD = N_A  # tiles shipping fp8 trans
N_BC = NTILES - N_A  # tiles shipping bf16 ln(trans)

# Tile visit order: column-chunk-major so each 2 MiB/4 n2 table chunk is
# needed just before its first tile. TILE_ORDER[k] = (i, jc) with jc the
# 1024-wide column block.
TILE_ORDER = [
    (i, jg2 * 2 + h) for jg2 in range(4) for i in range(NROW) for h in range(2)
]

# --- Schraudolph constants (path B: y ~ t*e^(c-1) * 2^S) ----------------
S2_SHIFT = 40.0
T_CLAMP = 1e-11
LOG2E = 1.4426950408889634
SCH_A = 1024.0 * LOG2E
SCH_B = 1024.0 * (S2_SHIFT + 15.0) - SCH_A  # bits = SCH_A*ps + SCH_B

T8_SCALE = 2.0**27


def _schraudolph_mean_ratio():
    """Value-weighted bias of the device Schraudolph path, Monte-Carlo'd
    with t ~ U(0,1)/N^2 (known) and cos ~ N(0, 1/16) using the exact op
    semantics (bf16 lnt, f32 affine, trunc to int16, bitcast f16).
    Used to unbias path-B partial sums."""
    rng = np.random.default_rng(7)
    n = 4_000_000
    t = rng.random(n, dtype=np.float32) / np.float32(NUM * NUM)
    c = np.clip(rng.normal(0, 1 / 16.0, n), -1, 1).astype(np.float32)
    lnt = np.log(np.maximum(t, T_CLAMP)).astype(NP_BF16).astype(np.float32)
    ps = lnt + c
    bits = ((np.float32(SCH_A) * ps + np.float32(SCH_B)).astype(np.float32)).astype(
        np.int16
    )
    y = bits.view(NP_F16).astype(np.float64)
    true = t.astype(np.float64) * np.exp(c.astype(np.float64) - 1.0)
    return float(y.sum() / (2.0**S2_SHIFT) / true.sum())


_cached = {}


def build_program():
    nc = bacc.Bacc(None, target_bir_lowering=False)

    idn = nc.declare_dram_parameter("idn", [128, 128], BF16, isOutput=False)
    n1d = nc.declare_dram_parameter("n1d", [2, 128, SHARD], FP8, isOutput=False)
    n2d = nc.declare_dram_parameter("n2d", [2, 128, NUM], FP8, isOutput=False)
    t8d = nc.declare_dram_parameter("t8d", [max(N_AD, 1), 128, TW], FP8, isOutput=False)
    lnd = nc.declare_dram_parameter("lnd", [N_BC, 128, TW], BF16, isOutput=False)
    acco = nc.declare_dram_parameter("acc", [128, NTILES], F32, isOutput=True)

    with tile.TileContext(nc) as tc:
        with (
            tc.tile_pool(name="const", bufs=1) as constp,
            tc.tile_pool(name="tabs", bufs=1) as tabp,
            tc.tile_pool(name="accp", bufs=1) as accp,
            tc.tile_pool(name="lntp", bufs=10) as lntp,
            tc.tile_pool(name="t8p", bufs=5) as t8p,
            tc.tile_pool(name="etp", bufs=4) as etp,
            tc.tile_pool(name="i16p", bufs=4) as i16p,
            tc.tile_pool(name="junk", bufs=1) as junkp,
            tc.tile_pool(name="psp", bufs=4, space="PSUM") as psp,
        ):
            ident = constp.tile([128, 128], BF16)
            nc.scalar.dma_start(out=ident[:], in_=idn[:, :])
            neg1 = constp.tile([128, 1], F32)
            nc.vector.memset(neg1[:], -1.0)

            n1s = tabp.tile([128, 2, SHARD], FP8)
            n2s = tabp.tile([128, 2, NUM], FP8)
            acc = accp.tile([128, NTILES], F32)

            junkb = junkp.tile([128, TW], BF16)  # ACT out, never read
            junka = junkp.tile([128, TW], BF16)  # DVE stt out, never read
            junkf = junkp.tile([128, 2 * TW], F16)  # Pool ts out, never read

            def load_n2_cols(c0, c1, eng=None):
                for kt in range(2):
                    (eng or nc.sync).dma_start(
                        out=n2s[:, kt, c0:c1], in_=n2d[kt, :, c0:c1]
                    )

            ia = 0  # index into t8d
            ibc = 0  # index into lnd
            for t in range(NTILES):
                i, jc = TILE_ORDER[t]
                n0 = jc * TW
                path = PATH[t]

                # data tile DMA first (so tile 0's data leads the queue)
                if path == "A":
                    t8 = t8p.tile([128, TW], FP8, tag="t8", name=f"t8_{t}")
                    nc.sync.dma_start(out=t8[:], in_=t8d[ia, :, :])
                    ia += 1
                else:
                    lt = lntp.tile([128, TW], BF16, tag="lnt", name=f"ln{t}")
                    nc.sync.dma_start(out=lt[:], in_=lnd[ibc, :, :])
                    ibc += 1

                if t == 0:
                    for kt in range(2):
                        nc.scalar.dma_start(out=n1s[:, kt, :], in_=n1d[kt, :, :])
                    load_n2_cols(0, TW)  # just the first tile's columns
                elif t == 1:
                    load_n2_cols(TW, CHUNK)  # rest of the first chunk
                if t % 16 == 5 and t // 16 < 3:
                    g = t // 16 + 1  # prefetch next column chunk
                    load_n2_cols(g * CHUNK, (g + 1) * CHUNK)

                ps = psp.tile([128, TW], F32, tag="ps", name=f"ps{t}")
                lhs = n1s[:, :, i * 128 : (i + 1) * 128]

                if path == "A":
                    for q in range(2):
                        c0 = q * 512
                        nc.tensor.matmul(
                            ps[:, c0 : c0 + 512],
                            lhsT=lhs,
                            rhs=n2s[:, :, n0 + c0 : n0 + c0 + 512],
                            perf_mode=mybir.MatmulPerfMode.DoubleRow,
                            start=True,
                            stop=True,
                            skip_group_check=True,
                        )
                    et = etp.tile([128, TW], BF16, tag="et", name=f"et{t}")
                    nc.scalar.activation(et[:], ps[:], AF.Exp, bias=neg1[:, 0:1])
                    nc.vector.scalar_tensor_tensor(
                        out=junka[:],
                        in0=et[:],
                        scalar=1.0,
                        in1=t8[:],
                        op0=ALU.mult,
                        op1=ALU.mult,
                        accum_out=acc[:, t : t + 1],
                    )
                else:
                    for q in range(2):
                        c0 = q * 512
                        nc.tensor.matmul(
                            ps[:, c0 : c0 + 512],
                            lhsT=ident[:],
                            rhs=lt[:, c0 : c0 + 512],
                            start=True,
                            stop=False,
                            skip_group_check=True,
                        )
                    for q in range(2):
                        c0 = q * 512
                        nc.tensor.matmul(
                            ps[:, c0 : c0 + 512],
                            lhsT=lhs,
                            rhs=n2s[:, :, n0 + c0 : n0 + c0 + 512],
                            perf_mode=mybir.MatmulPerfMode.DoubleRow,
                            start=False,
                            stop=True,
                            skip_group_check=True,
                        )
                    if path == "C":
                        nc.scalar.activation(
                            junkb[:],
                            ps[:],
                            AF.Exp,
                            bias=neg1[:, 0:1],
                            accum_out=acc[:, t : t + 1],
                        )
                    else:  # B: Schraudolph exp of (lnt + cos - 1) on DVE
                        i16 = i16p.tile([128, TW], I16, tag="i16", name=f"i16_{t}")
                        nc.vector.tensor_scalar(
                            out=i16[:],
                            in0=ps[:],
                            scalar1=SCH_A,
                            scalar2=SCH_B,
                            op0=ALU.mult,
                            op1=ALU.add,
                        )
                        nc.vector.tensor_scalar(
                            out=junkf[:, 0:TW],
                            in0=i16[:].bitcast(F16),
                            scalar1=1.0,
                            scalar2=0.0,
                            op0=ALU.mult,
                            op1=ALU.add,
                            accum_out=acc[:, t : t + 1],
                        )

            nc.sync.dma_start(out=acco[:, :], in_=acc[:])

    nc.finalize()
    return nc


def kernel(index1, index2, trans, emb1_w, emb2_w):
    # gather (identity for arange inputs, but stay correct in general)
    e1 = np.asarray(emb1_w, dtype=np.float32)[np.asarray(index1).astype(np.int64)]
    e2 = np.asarray(emb2_w, dtype=np.float32)[np.asarray(index2).astype(np.int64)]
    trans = np.ascontiguousarray(np.asarray(trans, dtype=np.float32))

    # ---- host: regularizer (exact) + sum(trans) ----------------------
    G1 = e1.T.astype(np.float64) @ e1.astype(np.float64)
    G2 = e2.T.astype(np.float64) @ e2.astype(np.float64)
    eye = np.eye(DIM, dtype=np.float64)
    reg = ((G1 - eye) ** 2).sum() + ((G2 - eye) ** 2).sum()
    st = float(trans.sum(dtype=np.float64))

    # ---- host: normalized, k-tiled transposed fp8 tables -------------
    n1 = e1 / np.sqrt((e1 * e1).sum(axis=1, keepdims=True))
    n2 = e2 / np.sqrt((e2 * e2).sum(axis=1, keepdims=True))
    n1T8 = np.ascontiguousarray(n1.T.reshape(2, 128, NUM).astype(NP_FP8))
    n2T8 = np.ascontiguousarray(n2.T.reshape(2, 128, NUM).astype(NP_FP8))

    # ---- host: per-tile trans encodings -------------------------------
    lnt_full = np.log(np.maximum(trans, T_CLAMP)).astype(NP_BF16)
    t8_full = (trans * np.float32(T8_SCALE)).astype(NP_FP8)

    if "nc" not in _cached:
        _cached["nc"] = build_program()
        _cached["ratio"] = _schraudolph_mean_ratio()
    nc = _cached["nc"]
    mean_r = _cached["ratio"]

    idn = np.eye(128, dtype=np.float32).astype(NP_BF16)
    in_maps = []
    for c in range(NCORES):
        r0 = c * SHARD
        t8_tiles = np.zeros((max(N_AD, 1), 128, TW), dtype=NP_FP8)
        ln_tiles = np.zeros((N_BC, 128, TW), dtype=NP_BF16)
        ia = ibc = 0
        for t in range(NTILES):
            i, jc = TILE_ORDER[t]
            rs = slice(r0 + i * 128, r0 + (i + 1) * 128)
            cs = slice(jc * TW, (jc + 1) * TW)
            if PATH[t] == "A":
                t8_tiles[ia] = t8_full[rs, cs]
                ia += 1
            else:
                ln_tiles[ibc] = lnt_full[rs, cs]
                ibc += 1
        in_maps.append(
            {
                "idn": idn,
                "n1d": np.ascontiguousarray(n1T8[:, :, r0 : r0 + SHARD]),
                "n2d": n2T8,
                "t8d": t8_tiles,
                "lnd": ln_tiles,
            }
        )

    res = run_bass_kernel_spmd(nc, in_maps, list(range(NCORES)))
    results = res.results

    syt = 0.0
    for c in range(NCORES):
        a = results[c]["acc"].astype(np.float64)  # [128, NTILES]
        for t in range(NTILES):
            s = a[:, t].sum()
            if PATH[t] == "A":
                syt += s / T8_SCALE
            elif PATH[t] == "B":
                syt += s / (2.0**S2_SHIFT) / mean_r
            else:
                syt += s

    d_w = st - syt
    return np.array([d_w, reg], dtype=np.float32)
